# revision 12
# baseline (speedup 1.0000x reference)
"""Euler integrator (low-rank quadratic Christoffel term) on 8 trn2 NeuronCores.

Math: per step   h = v @ U; gamma = (h*h) @ W; v' = v + dt*(force - gamma);
                 x' = wrap(x + dt*v)
Key reduction: the dynamics close in the rank-64 space:
    h_{t+1} = h_t + dt*(force@U) - (h_t^2) @ (dt*W@U)
    v_T = v_0 + T*dt*force - dt * A @ W,          A = sum_t h_t^2
    x_T = wrap(x_0 + T*dt*v_0 + 28*dt^2*force - dt^2 * G @ W),
                                                  G = sum_t (T-1-t) h_t^2
with T=8, and G == sum of A's running partial sums.  One wrap at the end
(mod-2pi commutes with addition), so HBM traffic is the 5-tensor roofline.

v3 structure: x/v/force host-packed into one [npack, 3, PACK, D] DRAM tensor
(ONE 3MB load per 1024-row pack, sync ring) and x'/v' into one
[npack, 2, PACK, D] output (ONE 2MB store, scalar ring).  ONE fused bf16
cast (v|f) and ONE batched [128, 4096] DMA-xbar transpose per pack (sync
ring) feed the rank-64 projection; the step loop runs entirely on full
128-contraction matmuls (block-diagonal -dt*W@U), with A and G both
accumulated on the tensor engine via (scaled) identity matmuls.  DVE does
only the exact fp32 +x0/+v0 adds and the magic-constant wrap.
"""

import sys

sys.path.insert(0, "/opt/trn_rl_repo")

import numpy as np
import ml_dtypes

import concourse.bacc as bacc
import concourse.mybir as mybir
import concourse.tile as tile
from concourse.tile_rust import add_dep_helper
from concourse.bass_utils import run_bass_kernel_spmd

F32 = mybir.dt.float32
BF16 = mybir.dt.bfloat16

DT = 0.01
PI = float(np.pi)
TWO_PI = 2.0 * PI
B, D, R = 262144, 256, 64
NCORES = 8
BL = B // NCORES          # rows per core
STEPS = 8
PACK = 1024               # batch rows per pack
NBLK = PACK // 128        # natural 128-row blocks per pack (8)
HN = 512                  # free size of h-space tiles (PACK/2)
MAGIC = 12582912.0        # 1.5 * 2**23 (fp32 RNE rounding trick)


def _chain(*insts):
    for a, b in zip(insts[1:], insts[:-1]):
        add_dep_helper(a.ins, b.ins, sync=True, reason="psum group order")


def _build(bl: int):
    npack = bl // PACK
    nc = bacc.Bacc("TRN2", target_bir_lowering=False, debug=False)

    xvf = nc.declare_dram_parameter("xvf", [npack, 3, PACK, D], F32, isOutput=False)
    # constants (host-prepared, tiny; all bf16 for single-pass matmuls)
    cdefs = {
        "u0z": 128, "u1z": 128,     # [U0|0], [U1|0]
        "u0": R, "u1": R,           # U halves
        "bdmd": 128,                # blockdiag(-dt*(W@U)) twice
        "wn": D, "wnn": D,          # -dt*W, -dt^2*W, dup'd on both halves
        "i128": 128,                # I_128 (A accumulation + fUdt adds)
        "if8": 128, "i28": 128,     # 8dt*I, 28dt^2*I
    }
    for t in range(STEPS - 1):
        cdefs[f"ig{t}"] = 128       # (7-t)*I for G accumulation on PE
    cdram = {
        nm: nc.declare_dram_parameter(nm, [128, w], BF16, isOutput=False)
        for nm, w in cdefs.items()
    }
    xvo = nc.declare_dram_parameter("xvo", [npack, 2, PACK, D], F32, isOutput=True)

    A = mybir.AluOpType

    with tile.TileContext(nc) as tc:
        with (
            tc.tile_pool(name="consts", bufs=1) as cpool,
            tc.tile_pool(name="ld", bufs=2) as ldp,
            tc.tile_pool(name="bf", bufs=2) as bfp,
            tc.tile_pool(name="tr", bufs=2) as trp,
            tc.tile_pool(name="hsp", bufs=3) as hsp,
            tc.tile_pool(name="atp", bufs=2) as atp,
            tc.tile_pool(name="outp", bufs=2) as outp,
            tc.tile_pool(name="wrapp", bufs=2) as wrapp,
            tc.tile_pool(name="ph", bufs=2, space="PSUM") as php,
            tc.tile_pool(name="pA", bufs=1, space="PSUM") as pAp,
            tc.tile_pool(name="pG", bufs=1, space="PSUM") as pGp,
            tc.tile_pool(name="pe", bufs=2, space="PSUM") as pep,
        ):
            cs = {}
            for nm, w in cdefs.items():
                t_ = cpool.tile([128, w], BF16, tag=nm)
                nc.sync.dma_start(out=t_[:], in_=cdram[nm][:])
                cs[nm] = t_

            for p in range(npack):
                # ---- ONE load for x, v, force (3MB), sync ring
                ld = ldp.tile([128, 3, NBLK, D], F32, tag="ld")
                nc.sync.dma_start(
                    out=ld[:],
                    in_=xvf[p].rearrange("t (n q) d -> q t n d", q=128),
                )
                # ONE fused bf16 cast of v|f (natural layout, contiguous)
                vfb = bfp.tile([128, 2, NBLK, D], BF16, tag="vfb")
                nc.scalar.copy(vfb[:], ld[:, 1:3])

                # ---- ONE batched xbar transpose [128, 4096] -> 32 blocks.
                # block t of vfT: tensor T=t//16, blk=(t%16)//2, dch=t%2:
                # vfT[p, t, k] = {v,f}[blk*128 + k, dch*128 + p]
                vfT = trp.tile([128, 4 * NBLK, 128], BF16, tag="vfT")
                nc.scalar.dma_start(out=vfT[:], in_=vfb[:], transpose=True)

                # ---- h0 / f@U into psum banks (strided interleaved rhs)
                # pf shares the pA pool's bank: it is fully consumed (fUdt
                # copy) before the A accumulation of the same pack starts.
                ph = php.tile([128, HN], F32, tag="ph")
                pf = pAp.tile([128, HN], F32, tag="pA")
                for bank, o in ((ph, 0), (pf, 16)):
                    _chain(
                        nc.tensor.matmul(
                            bank[:, :], cs["u0z"][:], vfT[:, o + 0:o + 8:2, :],
                            start=True, stop=False,
                        ),
                        nc.tensor.matmul(
                            bank[64:128, :], cs["u0"][:], vfT[:, o + 8:o + 16:2, :],
                            start=False, stop=False, skip_group_check=True,
                        ),
                        nc.tensor.matmul(
                            bank[64:128, :], cs["u1"][:], vfT[:, o + 9:o + 16:2, :],
                            start=False, stop=False, skip_group_check=True,
                        ),
                        nc.tensor.matmul(
                            bank[:, :], cs["u1z"][:], vfT[:, o + 1:o + 8:2, :],
                            start=False, stop=True,
                        ),
                    )
                fUdt = hsp.tile([128, HN], BF16, tag="fUdt")
                nc.scalar.mul(fUdt[:], pf[:], DT)

                # ---- step loop: squares on ACT; A, G, and h updates all
                #      as full-128-contraction accumulating matmuls.
                pA = pAp.tile([128, HN], F32, tag="pA")
                pG = pGp.tile([128, HN], F32, tag="pG")
                a_mms = []
                g_mms = []
                for t in range(STEPS):
                    hsq = hsp.tile([128, HN], BF16, tag="hsq")
                    nc.scalar.square(hsq[:], ph[:])
                    a_mms.append(nc.tensor.matmul(
                        pA[:, :], cs["i128"][:], hsq[:],
                        start=(t == 0), stop=(t == STEPS - 1),
                    ))
                    if t < STEPS - 1:
                        g_mms.append(nc.tensor.matmul(
                            pG[:, :], cs[f"ig{t}"][:], hsq[:],
                            start=(t == 0), stop=(t == STEPS - 2),
                        ))
                        nc.tensor.matmul(
                            ph[:, :], cs["i128"][:], fUdt[:],
                            start=False, stop=False, skip_group_check=True,
                        )
                        nc.tensor.matmul(
                            ph[:, :], cs["bdmd"][:], hsq[:],
                            start=False, stop=False, skip_group_check=True,
                        )
                _chain(*a_mms)
                _chain(*g_mms)
                At = atp.tile([128, HN], BF16, tag="At")
                Gt = atp.tile([128, HN], BF16, tag="Gt")
                nc.scalar.copy(At[:], pA[:])
                nc.scalar.copy(Gt[:], pG[:])

                # ---- epilogue (all 2D APs)
                xv = outp.tile([128, 2, NBLK, D], F32, tag="xv")

                for bg in range(4):      # 2 natural blocks per group
                    b0, b1 = bg * 2, bg * 2 + 2
                    pvf = pep.tile([128, 2 * D], F32, tag="pvf")
                    pxf = pep.tile([128, 2 * D], F32, tag="pxf")
                    vf_mms = []
                    xf_mms = []
                    for j in range(2):
                        blk = bg * 2 + j
                        half = blk // 4
                        hsl = slice(half * 64, (half + 1) * 64)
                        lsl = slice((blk % 4) * 128, (blk % 4) * 128 + 128)
                        osl = slice(j * D, (j + 1) * D)
                        vf_mms.append(nc.tensor.matmul(
                            pvf[:, osl], At[hsl, lsl], cs["wn"][hsl, :],
                            start=(j == 0), stop=False,
                        ))
                        xf_mms.append(nc.tensor.matmul(
                            pxf[:, osl], Gt[hsl, lsl], cs["wnn"][hsl, :],
                            start=(j == 0), stop=False,
                        ))
                    vf_mms.append(nc.tensor.matmul(
                        pvf[:, :], cs["if8"][:], vfb[:, 1, b0:b1, :],
                        start=False, stop=True,
                    ))
                    xf_mms.append(nc.tensor.matmul(
                        pxf[:, :], cs["if8"][:], vfb[:, 0, b0:b1, :],
                        start=False, stop=False,
                    ))
                    xf_mms.append(nc.tensor.matmul(
                        pxf[:, :], cs["i28"][:], vfb[:, 1, b0:b1, :],
                        start=False, stop=True,
                    ))
                    _chain(*vf_mms)
                    _chain(*xf_mms)

                    # vf = v0 + (8dt*force + A@Wn)     [exact fp32 add, DVE]
                    nc.vector.tensor_tensor(
                        xv[:, 1, b0:b1, :], ld[:, 1, b0:b1, :], pvf[:], A.add
                    )
                    # q = x0 + (8dt*v0 + 28dt^2*force + G@Wnn)
                    q = wrapp.tile([128, 2 * D], F32, tag="q")
                    nc.vector.tensor_tensor(q[:], ld[:, 0, b0:b1, :], pxf[:], A.add)
                    # wrap: r = RNE(q/2pi) via magic const; xf = q - 2pi*r
                    a1 = wrapp.tile([128, 2 * D], F32, tag="a1")
                    nc.vector.tensor_scalar(
                        a1[:], q[:], 1.0 / TWO_PI, MAGIC, A.mult, A.add,
                    )
                    rr = wrapp.tile([128, 2 * D], BF16, tag="rr")
                    nc.vector.tensor_scalar(
                        rr[:], a1[:], MAGIC, None, A.subtract,
                    )
                    nc.vector.scalar_tensor_tensor(
                        out=xv[:, 0, b0:b1, :], in0=rr[:],
                        scalar=-TWO_PI, in1=q[:], op0=A.mult, op1=A.add,
                    )

                # ---- ONE store for x', v' (2MB), scalar ring
                nc.scalar.dma_start(
                    out=xvo[p].rearrange("t (n q) d -> q t n d", q=128),
                    in_=xv[:],
                )

    nc.compile()
    return nc


_NC_CACHE = {}


def _get_nc(bl: int):
    if bl not in _NC_CACHE:
        _NC_CACHE[bl] = _build(bl)
    return _NC_CACHE[bl]


def _consts(U, W):
    U32 = np.ascontiguousarray(U, dtype=np.float32)
    W32 = np.ascontiguousarray(W, dtype=np.float32)
    bf = ml_dtypes.bfloat16
    dup = lambda a: np.concatenate([a, a], axis=0)
    md = -(DT * (W32 @ U32))
    eye = np.eye(128, dtype=np.float32)
    z = np.zeros((128, 64), np.float32)
    zr = np.zeros((R, R), np.float32)
    out = {
        "u0z": np.concatenate([U32[:128, :], z], axis=1).astype(bf),
        "u1z": np.concatenate([U32[128:, :], z], axis=1).astype(bf),
        "u0": U32[:128, :].astype(bf),
        "u1": U32[128:, :].astype(bf),
        "bdmd": np.block([[md, zr], [zr, md]]).astype(bf),
        "wn": dup(-DT * W32).astype(bf),
        "wnn": dup(-DT * DT * W32).astype(bf),
        "i128": eye.astype(bf),
        "if8": ((8.0 * DT) * eye).astype(bf),
        "i28": ((28.0 * DT * DT) * eye).astype(bf),
    }
    for t in range(STEPS - 1):
        out[f"ig{t}"] = (float(STEPS - 1 - t) * eye).astype(bf)
    return out


def kernel(x, v, force, U, W, steps=STEPS, **_ignored):
    assert int(steps) == STEPS, f"kernel hardcodes steps={STEPS}, got {steps}"
    x = np.asarray(x, dtype=np.float32)
    v = np.asarray(v, dtype=np.float32)
    force = np.asarray(force, dtype=np.float32)
    consts = _consts(U, W)

    nc = _get_nc(BL)
    npack = BL // PACK
    in_maps = []
    for i in range(NCORES):
        sl = slice(i * BL, (i + 1) * BL)
        # [3, BL, D] -> [npack, 3, PACK, D]: per-pack interleave so each
        # pack's x/v/f rows are one 3D-balanceable DMA.
        stk = np.stack([x[sl], v[sl], force[sl]])
        stk = np.ascontiguousarray(
            stk.reshape(3, npack, PACK, D).transpose(1, 0, 2, 3)
        )
        m = {"xvf": stk}
        m.update(consts)
        in_maps.append(m)

    res = run_bass_kernel_spmd(nc, in_maps, core_ids=list(range(NCORES)))
    xf = np.empty((B, D), np.float32)
    vf = np.empty((B, D), np.float32)
    for i in range(NCORES):
        out = res.results[i]["xvo"]         # [npack, 2, PACK, D]
        sl = slice(i * BL, (i + 1) * BL)
        xf[sl] = out[:, 0].reshape(BL, D)
        vf[sl] = out[:, 1].reshape(BL, D)
    return (xf, vf)
